# revision 1
# baseline (speedup 1.0000x reference)
"""Single-head attention (B=8, S=2048, D=128) on 8 Trainium2 NeuronCores.

Sharding: data-parallel over batch — core b computes batch element b end to end
(no collectives). kernel() takes full inputs, returns the full output.

Per-core algorithm (Tile framework, one NEFF run SPMD on 8 cores):
  - x is DMA'd with 16 consecutive rows per partition (8 KB contiguous per
    partition, near-peak DMA). This perfectly-shuffles the sequence axis
    (s = 16p + t); attention is permutation-equivariant, so the output DMA
    simply inverts the shuffle.
  - xT = PE-transpose(x tiles); QT/KT = W.T @ xT + b as [d,s] (bf16), V
    directly as [s,d] tiles from xT (bf16, bias via broadcast add).
  - Main loop, software-pipelined over 8 chunks (2 k-tiles) per q-group:
      scoresT[sk,sq] = KT_kt.T @ QT_g   (bf16 matmuls, N=512, psum fp32,
                                         double-buffered 2-bank stage slots)
      PT = exp(scale*scoresT)           (one ScalarE activation per chunk,
                                         psum->sbuf, bf16 out)
      oT += V_kt.T' @ PT                (AV accumulate [d,sq] in psum)
      den: ones.T @ PT                  (4 M=32 col-group-packed matmuls per
                                         2 chunks, concurrent in PE array)
    PE issues scores one chunk ahead of AV/den so it never head-of-line
    blocks on the exp.
  - Epilogue per group: den strips -> sbuf, selector matmuls (sum strips AND
    put q on partitions), one reciprocal [128,16], PE-transpose of oT back to
    [sq,d], per-partition scale by 1/den during the psum->sbuf copy, DMA out.

Numerics: scores/AV in bf16 with fp32 psum accumulation (rel err ~2.6e-3 vs
fp32 reference; exp/softmax denominators in fp32, den reduction in fp32r).
Set USE_BF16_QK=False for fp32r (~fp22) scores (~9e-4 rel err, ~15 us slower).
"""

import numpy as np

S = 2048
D = 128
USE_BF16_QK = True
NT = S // 128          # 16 s-tiles of 128
NG = S // 512          # 4 q-groups of 512
SCALE = float(1.0 / np.sqrt(D))

_PROGRAM = None
LAST_RESULTS = None


def _build():
    from contextlib import ExitStack

    import concourse.bass as bass
    import concourse.mybir as mybir
    import concourse.tile as tile
    from concourse import bacc

    fp32 = mybir.dt.float32
    fp32r = mybir.dt.float32r
    bf16 = mybir.dt.bfloat16
    qkdt = bf16 if USE_BF16_QK else fp32r
    Exp = mybir.ActivationFunctionType.Exp

    nc = bacc.Bacc(trn_type="TRN2", target_bir_lowering=False)

    x_d = nc.dram_tensor("x", [S, D], fp32, kind="ExternalInput").ap()
    w_d = nc.dram_tensor("w3", [D, 3 * D], fp32, kind="ExternalInput").ap()
    # consts layout: [bq | bk | bv | sel(4) | ident(128) | bv_bcast(4x128)] = [128, 647]
    c_d = nc.dram_tensor("consts", [D, 647], fp32, kind="ExternalInput").ap()
    out_d = nc.dram_tensor("out", [S, D], fp32, kind="ExternalOutput").ap()

    # x loaded with 16 consecutive rows per partition (8 KB contiguous per
    # partition -> near-peak DMA). This applies the perfect-shuffle permutation
    # s = 16*p + t to the sequence axis; attention is permutation-equivariant,
    # so we simply invert it when storing the output.
    x_r = x_d.rearrange("(p r) d -> p r d", p=128)
    out_r = out_d.rearrange("(p r) d -> p r d", p=128)

    with tile.TileContext(nc) as tc, ExitStack() as ctx:
        singles = ctx.enter_context(tc.tile_pool(name="singles", bufs=1))
        xin = ctx.enter_context(tc.tile_pool(name="xin", bufs=3))
        ptp = ctx.enter_context(tc.tile_pool(name="pt", bufs=4))
        outp = ctx.enter_context(tc.tile_pool(name="outp", bufs=2))
        # PSUM: stage 4 banks + av 2 + tp/den 2 = 8 banks exactly
        stage_p = ctx.enter_context(tc.tile_pool(name="stage", bufs=1, space="PSUM"))
        av_p = ctx.enter_context(tc.tile_pool(name="av", bufs=1, space="PSUM"))
        tp_p = ctx.enter_context(tc.tile_pool(name="tp", bufs=1, space="PSUM"))

        # --- constants (small consts DMA first: identity gates the transposes) ---
        consts_sb = singles.tile([128, 647], fp32, tag="consts")
        nc.sync.dma_start(out=consts_sb[:, 0:135], in_=c_d[:, 0:135])
        nc.gpsimd.dma_start(out=consts_sb[:, 135:647], in_=c_d[:, 135:647])
        bq_sb = consts_sb[:, 0:1]
        bk_sb = consts_sb[:, 1:2]
        bv_sb = consts_sb[:, 2:3]
        id_sb = consts_sb[:, 7:135]
        bvb_sb = consts_sb[:, 135:647]
        sel_sb = singles.tile([128, 4], fp32r, tag="sel")
        nc.vector.tensor_copy(sel_sb, consts_sb[:, 3:7])
        ones_sb = singles.tile([128, 128], bf16, tag="ones")
        id16_sb = singles.tile([128, 128], bf16, tag="id16")
        ones_stage = xin.tile([128, 128], fp32, tag="wstage")
        nc.vector.memset(ones_stage, 1.0)
        nc.vector.tensor_copy(ones_sb, ones_stage)
        nc.vector.tensor_copy(id16_sb, id_sb)

        # x: 4 quarter-DMAs (2 KB contiguous per partition each) so the
        # transfers spread over multiple HWDGE queues in parallel
        x_q = []
        for h in range(4):
            xh = singles.tile([128, 4, 128], fp32, tag=f"xh{h}", name=f"xh_{h}")
            nc.sync.dma_start(out=xh, in_=x_r[:, 4 * h:4 * (h + 1), :])
            x_q.append(xh)

        w3_stage = singles.tile([128, 384], fp32, tag="w3stage")
        nc.gpsimd.dma_start(out=w3_stage, in_=w_d)
        w3_sb = singles.tile([128, 384], qkdt, tag="w3")
        nc.vector.tensor_copy(w3_sb, w3_stage)
        wq_sb = w3_sb[:, 0:128]
        wk_sb = w3_sb[:, 128:256]
        wv_sb = w3_sb[:, 256:384]

        # --- persistent big sbuf tensors ---
        xT_sb = singles.tile([128, S], qkdt, tag="xT")   # [d, s]
        qT_sb = singles.tile([128, S], qkdt, tag="qT")   # [e, s]
        kT_sb = singles.tile([128, S], qkdt, tag="kT")   # [e, s]
        v_sb = singles.tile([128, S], bf16, tag="v")     # 16 tiles of [s(128), d]

        # per chunk-of-4-tiles: cast x to bf16, transpose, project QT/KT, compute V
        for c in range(4):
            tpt = tp_p.tile([128, 512], fp32, tag=f"tp{c % 2}", name=f"tptx_{c}")
            for j in range(4):
                t = 4 * c + j
                nc.tensor.matmul(
                    tpt[:, 128 * j:128 * (j + 1)], lhsT=x_q[t // 4][:, t % 4, :],
                    rhs=id_sb,
                    is_transpose=True, start=(j == 0), stop=(j == 3),
                )
            nc.vector.tensor_copy(xT_sb[:, 512 * c:512 * (c + 1)], tpt)
        for c in range(4):
            sl = slice(512 * c, 512 * (c + 1))
            for wi, (w_sb, b_sb, dst) in enumerate((
                (wq_sb, bq_sb, qT_sb), (wk_sb, bk_sb, kT_sb),
            )):
                pp = av_p.tile([128, 512], fp32, tag=f"av{wi % 2}", name=f"pp_{c}_{wi}")
                nc.tensor.matmul(pp, lhsT=w_sb, rhs=xT_sb[:, sl],
                                 start=True, stop=True)
                nc.vector.tensor_scalar_add(dst[:, sl], pp, b_sb)
        for c in range(4):
            tpv = tp_p.tile([128, 512], fp32, tag=f"tp{(c + 1) % 2}", name=f"tptv_{c}")
            for j in range(4):
                t = 4 * c + j
                nc.tensor.matmul(
                    tpv[:, 128 * j:128 * (j + 1)],
                    lhsT=xT_sb[:, 128 * t:128 * (t + 1)], rhs=wv_sb,
                    start=(j == 0), stop=(j == 3), skip_group_check=True,
                )
            nc.vector.tensor_add(v_sb[:, 512 * c:512 * (c + 1)], tpv, bvb_sb)

        # --- main attention loop, software-pipelined over 2-k-tile chunks ---
        # Per step: PE issues scores(chunk i) first (stage is double-buffered,
        # so always ready), then AV+den for chunk i-1 (whose exp finished during
        # the previous step). ScalarE exp of chunk i overlaps AV/den of i-1.
        NCH = 8                       # chunks per group, 2 k-tiles each
        chunks = [(g, c) for g in range(NG) for c in range(NCH)]
        av = den = None
        avs, dens, pts = {}, {}, {}

        def issue_scores(g, c):
            st = stage_p.tile([128, 1024], fp32, tag=f"stage{(g * NCH + c) % 2}",
                              name=f"st_{g}_{c}")
            with nc.named_scope("scores"):
                for j in range(2):
                    kt = 2 * c + j
                    nc.tensor.matmul(
                        st[:, 512 * j:512 * (j + 1)],
                        lhsT=kT_sb[:, 128 * kt:128 * (kt + 1)],
                        rhs=qT_sb[:, 512 * g:512 * (g + 1)],
                        start=True, stop=True,
                    )
            pt = ptp.tile([128, 1024], bf16, tag=f"pt{(g * NCH + c) % 2}", name=f"pt_{g}_{c}", bufs=2)
            with nc.named_scope("exp"):
                nc.scalar.activation(pt, st, Exp, scale=SCALE)
            return pt

        def issue_den_quad(g, c0):
            # quad covers chunks c0, c0+1 (k-tiles 2*c0 .. 2*c0+3), both pt
            # tiles already materialized -> 4 back-to-back col-group matmuls
            # run concurrently in the PE array.
            with nc.named_scope("den"):
                for q in range(4):
                    kt = 2 * c0 + q
                    ptq, jq = pts[g, c0 + q // 2], kt % 2
                    strip = kt % 4
                    nc.tensor.matmul(
                        dens[g][32 * strip:32 * (strip + 1), :],
                        lhsT=ones_sb[:, 0:32],
                        rhs=ptq[:, 512 * jq:512 * (jq + 1)],
                        start=(c0 == 0), stop=(c0 == NCH - 2),
                        tile_position=(0, 32 * strip),
                        skip_group_check=True,
                    )

        def issue_avden(g, c, pt):
            pts[g, c] = pt
            with nc.named_scope("av"):
                for j in range(2):
                    kt = 2 * c + j
                    nc.tensor.matmul(
                        avs[g], lhsT=v_sb[:, 128 * kt:128 * (kt + 1)],
                        rhs=pt[:, 512 * j:512 * (j + 1)],
                        start=(kt == 0), stop=(kt == 15),
                    )
            if c % 2 == 0 and c > 0:
                issue_den_quad(g, c - 2)

        def epilogue(g):
            av, den = avs.pop(g), dens.pop(g)
            with nc.named_scope("epi"):
                den_fs = outp.tile([128, 512], fp32r, tag=f"denfs{g % 2}", name=f"denfs_{g}", bufs=1)
                nc.vector.tensor_copy(den_fs, den)
                denT = tp_p.tile([128, 16], fp32, tag=f"tp{(g + 1) % 2}", name=f"denT_{g}")
                for j in range(4):
                    nc.tensor.matmul(
                        denT[:, 4 * j:4 * (j + 1)],
                        lhsT=den_fs[:, 128 * j:128 * (j + 1)],
                        rhs=sel_sb, start=(j == 0), stop=(j == 3),
                    )
                recip = outp.tile([128, 16], fp32, tag=f"recip{g % 2}", name=f"recip_{g}", bufs=1)
                nc.vector.reciprocal(recip, denT)
                oT_sb = outp.tile([128, 512], bf16, tag=f"oTsb{g % 2}", name=f"oTsb_{g}", bufs=1)
                nc.vector.tensor_copy(oT_sb, av)
                tpo = tp_p.tile([128, 512], bf16, tag=f"tp{g % 2}", name=f"tpo_{g}")
                for j in range(4):
                    nc.tensor.matmul(
                        tpo[:, 128 * j:128 * (j + 1)],
                        lhsT=oT_sb[:, 128 * j:128 * (j + 1)], rhs=id16_sb,
                        is_transpose=True, start=(j == 0), stop=(j == 3),
                    )
                osb = outp.tile([128, 512], fp32, tag=f"osb{g % 2}", name=f"osb_{g}", bufs=1)
                for j in range(4):
                    nc.vector.tensor_scalar_mul(
                        osb[:, 128 * j:128 * (j + 1)],
                        tpo[:, 128 * j:128 * (j + 1)], recip[:, 4 * j:4 * j + 1],
                    )
                nc.sync.dma_start(
                    out=out_r[:, 4 * g:4 * (g + 1), :],
                    in_=osb.rearrange("p (j d) -> p j d", j=4),
                )

        prev = None
        for g, c in chunks:
            if c == 0:
                avs[g] = av_p.tile([128, 512], fp32, tag=f"av{g % 2}", name=f"av_{g}")
                dens[g] = tp_p.tile([128, 512], fp32, tag=f"tp{g % 2}", name=f"den_{g}")
            pt = issue_scores(g, c)
            if prev is not None:
                issue_avden(*prev)
                if prev[1] == NCH - 1:
                    issue_den_quad(prev[0], NCH - 2)
                    epilogue(prev[0])
            prev = (g, c, pt)
        issue_avden(*prev)
        issue_den_quad(prev[0], NCH - 2)
        epilogue(prev[0])

    nc.compile()
    return nc


def _get_program():
    global _PROGRAM
    if _PROGRAM is None:
        _PROGRAM = _build()
    return _PROGRAM


def _ensure_axon_hooks():
    """bass_utils imports antenv.axon_hooks when tracing; provide a stub if
    the image's antenv lacks it (hook defaults to None => tracing skipped)."""
    import sys
    import types
    try:
        import antenv.axon_hooks  # noqa: F401
        return
    except ImportError:
        pass
    import antenv
    m = types.ModuleType("antenv.axon_hooks")
    m._hook = None
    def _set(h):
        m._hook = h
    def _get():
        return m._hook
    m.set_axon_ntff_profile_hook = _set
    m.get_axon_ntff_profile_hook = _get
    sys.modules["antenv.axon_hooks"] = m
    antenv.axon_hooks = m


def kernel(input1, Wq, bq, Wk, bk, Wv, bv):
    global LAST_RESULTS
    _ensure_axon_hooks()
    from concourse.bass_utils import run_bass_kernel_spmd

    nc = _get_program()

    input1 = np.ascontiguousarray(np.asarray(input1, dtype=np.float32))
    w3 = np.concatenate([np.asarray(W, np.float32).T for W in (Wq, Wk, Wv)],
                        axis=1)
    sel = np.tile(np.array([1.0 if p % 32 == 0 else 0.0 for p in range(D)],
                  np.float32).reshape(D, 1), (1, 4))
    consts = np.concatenate([
        np.asarray(bq, np.float32).reshape(D, 1),
        np.asarray(bk, np.float32).reshape(D, 1),
        np.asarray(bv, np.float32).reshape(D, 1),
        sel,
        np.eye(D, dtype=np.float32),
        np.tile(np.asarray(bv, np.float32).reshape(1, D), (D, 4)),
    ], axis=1)
    common = {
        "w3": np.ascontiguousarray(w3),
        "consts": np.ascontiguousarray(consts),
    }
    in_maps = [dict(common, x=input1[b]) for b in range(8)]
    res = run_bass_kernel_spmd(nc, in_maps, core_ids=list(range(8)))
    LAST_RESULTS = res
    return np.stack([r["out"] for r in res.results], axis=0)



# revision 6
# speedup vs baseline: 1.0593x; 1.0593x over previous
"""Single-head attention (B=8, S=2048, D=128) on 8 Trainium2 NeuronCores.

Sharding: data-parallel over batch - core b computes batch element b end to end
(no collectives). kernel() takes full inputs, returns the full output.

v2 design notes (per core):
  - Host casts x and [Wq.T|Wk.T|Wv.T] to bf16 (the compute is bf16 anyway),
    halving input DMA and removing all fp32->bf16 prologue casts. Output is
    DMA'd bf16 and widened to fp32 on host.
  - bk is dropped: softmax over keys is invariant to a per-query additive
    shift, and (Q)(K0+bk)^T - (Q)(K0)^T = (Q bk) 1^T is constant per query.
  - x is DMA'd with 16 consecutive rows per partition (perfect shuffle
    s = 16p + t of the sequence axis; attention is permutation-equivariant,
    the output DMA inverts it) in 4 prioritized slices on different engine
    queues; slice-s prologue (transpose, K/Q/V projections) is emitted
    interleaved with main-loop chunks so compute starts after slice 0 only.
  - PSUM budget: scores stage 3 slots x 2 banks (prologue scratch shares the
    slot rotation), AV accumulator 1 bank (single-buffered; the psum->sbuf
    copy is the first epilogue op and gates the next group's first AV), den
    1 bank (hosts den -> denT -> tpo sequentially via same-tag rotation).
  - Main loop per chunk (2 k-tiles x 512 q): scoresT = kT.T @ qT (2 bf16
    matmuls N=512, fp32 psum), one ScalarE exp [128,1024] psum->sbuf bf16,
    AV accumulate oT += v.T @ P (2 matmuls), den via M=32 col-packed ones
    matmuls every 2 chunks. Cadence is exp-bound (~1.12us).
  - Epilogue per group: av copy to sbuf, den strips -> selector matmuls ->
    reciprocal, PE transpose back to [q, d], per-partition 1/den scale fused
    with the psum->sbuf copy, bf16 DMA out.
"""

import numpy as np

S = 2048
D = 128
NT = S // 128          # 16 s-tiles of 128
NG = S // 512          # 4 q-groups of 512
NCH = 8                # chunks per group, 2 k-tiles each
SCALE = float(1.0 / np.sqrt(D))

_PROGRAM = None
LAST_RESULTS = None


def _build():
    from contextlib import ExitStack

    import concourse.bass as bass
    import concourse.mybir as mybir
    import concourse.tile as tile
    from concourse import bacc

    fp32 = mybir.dt.float32
    fp32r = mybir.dt.float32r
    bf16 = mybir.dt.bfloat16
    Exp = mybir.ActivationFunctionType.Exp

    nc = bacc.Bacc(trn_type="TRN2", target_bir_lowering=False)

    x_d = nc.dram_tensor("x", [S, D], bf16, kind="ExternalInput").ap()
    w_d = nc.dram_tensor("w3", [D, 3 * D], bf16, kind="ExternalInput").ap()
    # bf16 consts: [ident(128) | ones(32)]
    cb_d = nc.dram_tensor("cb", [D, 160], bf16, kind="ExternalInput").ap()
    # fp32 consts: [bq(1) | sel(4) | bv_bcast(4x128)]
    cf_d = nc.dram_tensor("cf", [D, 517], fp32, kind="ExternalInput").ap()
    out_d = nc.dram_tensor("out", [S, D], bf16, kind="ExternalOutput").ap()

    x_r = x_d.rearrange("(p r) d -> p r d", p=128)
    out_r = out_d.rearrange("(p r) d -> p r d", p=128)

    with tile.TileContext(nc) as tc, ExitStack() as ctx:
        singles = ctx.enter_context(tc.tile_pool(name="singles", bufs=1))
        ptp = ctx.enter_context(tc.tile_pool(name="ptp", bufs=1))
        outp = ctx.enter_context(tc.tile_pool(name="outp", bufs=1))
        # PSUM: stage 3 slots x 2 banks + av 1 + den 1 = 8 banks exactly
        stage_p = ctx.enter_context(tc.tile_pool(name="stage", bufs=1, space="PSUM"))
        av_p = ctx.enter_context(tc.tile_pool(name="av", bufs=1, space="PSUM"))
        den_p = ctx.enter_context(tc.tile_pool(name="den", bufs=1, space="PSUM"))

        # --- input DMAs, priority order: cb (gates transposes), w3, x slice 0 ---
        cb_sb = singles.tile([128, 160], bf16, tag="cb")
        nc.sync.dma_start(out=cb_sb, in_=cb_d)
        id_sb = cb_sb[:, 0:128]
        ones_sb = cb_sb[:, 128:160]

        w3_sb = singles.tile([128, 384], bf16, tag="w3")
        nc.gpsimd.dma_start(out=w3_sb, in_=w_d)
        wq_sb = w3_sb[:, 0:128]
        wk_sb = w3_sb[:, 128:256]
        wv_sb = w3_sb[:, 256:384]

        x_sl = []
        dma_eng = [nc.sync, nc.gpsimd, nc.scalar, nc.sync]
        for h in range(4):
            xh = singles.tile([128, 4, 128], bf16, tag=f"xh{h}", name=f"xh_{h}")
            dma_eng[h].dma_start(out=xh, in_=x_r[:, 4 * h:4 * (h + 1), :])
            x_sl.append(xh)

        cf_sb = singles.tile([128, 517], fp32, tag="cf")
        nc.gpsimd.dma_start(out=cf_sb, in_=cf_d)
        bq_sb = cf_sb[:, 0:1]
        bvb_sb = cf_sb[:, 5:517]
        sel_sb = singles.tile([128, 4], fp32r, tag="sel")
        nc.vector.tensor_copy(sel_sb, cf_sb[:, 1:5])

        # --- persistent big sbuf tensors ---
        xT_sb = singles.tile([128, S], bf16, tag="xT")   # [d, s]
        qT_sb = singles.tile([128, S], bf16, tag="qT")   # [e, s]
        kT_sb = singles.tile([128, S], bf16, tag="kT")   # [e, s]
        v_sb = singles.tile([128, S], bf16, tag="v")     # 16 tiles of [s(128), d]

        stage_i = [0]

        def stage_tile(shape, dtype, name):
            # 3 rotating slots; tag size is the max over tiles (the [128,1024]
            # fp32 scores stage) so every slot is 2 PSUM banks.
            t = stage_p.tile(shape, dtype, tag=f"s{stage_i[0] % 3}", name=name)
            stage_i[0] += 1
            return t

        def prologue_slice(s, scalar_eng):
            """Transpose x slice s, project kT/qT/v. scalar_eng: route the kT
            and qT psum->sbuf ops to ScalarE (only for slice 0, pre-main)."""
            sl = slice(512 * s, 512 * (s + 1))
            tpx = stage_tile([128, 512], bf16, f"tpx_{s}")
            for j in range(4):
                nc.tensor.matmul(
                    tpx[:, 128 * j:128 * (j + 1)], lhsT=x_sl[s][:, j, :],
                    rhs=id_sb, is_transpose=True, start=(j == 0), stop=(j == 3),
                )
            nc.vector.tensor_copy(xT_sb[:, sl], tpx)

            pk = stage_tile([128, 512], fp32, f"pk_{s}")
            nc.tensor.matmul(pk, lhsT=wk_sb, rhs=xT_sb[:, sl], start=True, stop=True)
            if scalar_eng:
                nc.scalar.copy(kT_sb[:, sl], pk)
            else:
                nc.vector.tensor_copy(kT_sb[:, sl], pk)

            pq = stage_tile([128, 512], fp32, f"pq_{s}")
            nc.tensor.matmul(pq, lhsT=wq_sb, rhs=xT_sb[:, sl], start=True, stop=True)
            nc.vector.tensor_scalar_add(qT_sb[:, sl], pq, bq_sb)

            pv = stage_tile([128, 512], fp32, f"pv_{s}")
            for j in range(4):
                t = 4 * s + j
                nc.tensor.matmul(
                    pv[:, 128 * j:128 * (j + 1)],
                    lhsT=xT_sb[:, 128 * t:128 * (t + 1)], rhs=wv_sb,
                    start=(j == 0), stop=(j == 3), skip_group_check=True,
                )
            nc.vector.tensor_add(v_sb[:, sl], pv, bvb_sb)

        # --- main attention loop, software-pipelined over 2-k-tile chunks ---
        avs, dens, pts = {}, {}, {}

        def issue_scores(g, c):
            st = stage_tile([128, 1024], fp32, f"st_{g}_{c}")
            with nc.named_scope("scores"):
                for j in range(2):
                    kt = 2 * c + j
                    nc.tensor.matmul(
                        st[:, 512 * j:512 * (j + 1)],
                        lhsT=kT_sb[:, 128 * kt:128 * (kt + 1)],
                        rhs=qT_sb[:, 512 * g:512 * (g + 1)],
                        start=True, stop=True,
                    )
            pt = ptp.tile([128, 1024], bf16, tag="pt", name=f"pt_{g}_{c}", bufs=6)
            with nc.named_scope("exp"):
                nc.scalar.activation(pt, st, Exp, scale=SCALE)
            return pt

        def issue_den_quad(g, c0):
            # quad covers chunks c0, c0+1 (k-tiles 2*c0 .. 2*c0+3): 4
            # back-to-back M=32 col-group matmuls run concurrently in the PE.
            # The den tile is allocated at the first quad so the single-buffer
            # rotation order is den_g, denT_g, tpo_g, den_{g+1}, ...
            if c0 == 0:
                dens[g] = den_p.tile([128, 512], fp32, tag="den", name=f"den_{g}")
            with nc.named_scope("den"):
                for q in range(4):
                    kt = 2 * c0 + q
                    ptq, jq = pts[g, c0 + q // 2], kt % 2
                    strip = kt % 4
                    nc.tensor.matmul(
                        dens[g][32 * strip:32 * (strip + 1), :],
                        lhsT=ones_sb,
                        rhs=ptq[:, 512 * jq:512 * (jq + 1)],
                        start=(c0 == 0), stop=(c0 == NCH - 2),
                        tile_position=(0, 32 * strip),
                        skip_group_check=True,
                    )

        def issue_avden(g, c, pt):
            pts[g, c] = pt
            with nc.named_scope("av"):
                for j in range(2):
                    kt = 2 * c + j
                    nc.tensor.matmul(
                        avs[g], lhsT=v_sb[:, 128 * kt:128 * (kt + 1)],
                        rhs=pt[:, 512 * j:512 * (j + 1)],
                        start=(kt == 0), stop=(kt == 15),
                    )
            if c % 2 == 0 and c > 0:
                issue_den_quad(g, c - 2)

        def epilogue(g):
            av, den = avs.pop(g), dens.pop(g)
            last = (g == NG - 1)
            with nc.named_scope("epi"):
                # av copy first: frees the single av bank for group g+1
                oT_sb = outp.tile([128, 512], bf16, tag="oTsb", name=f"oTsb_{g}",
                                  bufs=2)
                den_fs = outp.tile([128, 512], fp32r, tag="denfs", name=f"denfs_{g}",
                                   bufs=2)
                if last:
                    nc.vector.tensor_copy(den_fs, den)
                    nc.vector.tensor_copy(oT_sb, av)
                else:
                    nc.vector.tensor_copy(oT_sb, av)
                    nc.vector.tensor_copy(den_fs, den)
                denT = den_p.tile([128, 16], fp32, tag="den", name=f"denT_{g}")
                for j in range(4):
                    nc.tensor.matmul(
                        denT[:, 4 * j:4 * (j + 1)],
                        lhsT=den_fs[:, 128 * j:128 * (j + 1)],
                        rhs=sel_sb, start=(j == 0), stop=(j == 3),
                    )
                recip = outp.tile([128, 16], fp32, tag="recip", name=f"recip_{g}",
                                  bufs=2)
                nc.vector.reciprocal(recip, denT)
                tpo = den_p.tile([128, 512], bf16, tag="den", name=f"tpo_{g}")
                for j in range(4):
                    nc.tensor.matmul(
                        tpo[:, 128 * j:128 * (j + 1)],
                        lhsT=oT_sb[:, 128 * j:128 * (j + 1)], rhs=id_sb,
                        is_transpose=True, start=(j == 0), stop=(j == 3),
                    )
                osb = outp.tile([128, 512], bf16, tag="osb", name=f"osb_{g}", bufs=2)
                for j in range(4):
                    nc.vector.tensor_scalar_mul(
                        osb[:, 128 * j:128 * (j + 1)],
                        tpo[:, 128 * j:128 * (j + 1)], recip[:, 4 * j:4 * j + 1],
                    )
                nc.sync.dma_start(
                    out=out_r[:, 4 * g:4 * (g + 1), :],
                    in_=osb.rearrange("p (j d) -> p j d", j=4),
                )

        prologue_slice(0, scalar_eng=True)
        prologue_at = {(0, 2): 1, (0, 4): 2, (0, 6): 3}

        chunks = [(g, c) for g in range(NG) for c in range(NCH)]
        prev = None
        for g, c in chunks:
            if (g, c) in prologue_at:
                prologue_slice(prologue_at[(g, c)], scalar_eng=False)
            if c == 0:
                avs[g] = av_p.tile([128, 512], fp32, tag="av", name=f"av_{g}")
            pt = issue_scores(g, c)
            if prev is not None:
                issue_avden(*prev)
                if prev[1] == NCH - 1:
                    issue_den_quad(prev[0], NCH - 2)
                    epilogue(prev[0])
            prev = (g, c, pt)
        issue_avden(*prev)
        issue_den_quad(prev[0], NCH - 2)
        epilogue(prev[0])

    nc.compile()
    return nc


def _get_program():
    global _PROGRAM
    if _PROGRAM is None:
        _PROGRAM = _build()
    return _PROGRAM


def _ensure_axon_hooks():
    """bass_utils imports antenv.axon_hooks when tracing; provide a stub if
    the image's antenv lacks it (hook defaults to None => tracing skipped)."""
    import sys
    import types
    try:
        import antenv.axon_hooks  # noqa: F401
        return
    except ImportError:
        pass
    import antenv
    m = types.ModuleType("antenv.axon_hooks")
    m._hook = None
    def _set(h):
        m._hook = h
    def _get():
        return m._hook
    m.set_axon_ntff_profile_hook = _set
    m.get_axon_ntff_profile_hook = _get
    sys.modules["antenv.axon_hooks"] = m
    antenv.axon_hooks = m


def kernel(input1, Wq, bq, Wk, bk, Wv, bv):
    global LAST_RESULTS
    _ensure_axon_hooks()
    import ml_dtypes
    from concourse.bass_utils import run_bass_kernel_spmd

    nc = _get_program()
    bft = ml_dtypes.bfloat16

    input1 = np.asarray(input1, dtype=np.float32)
    w3 = np.concatenate([np.asarray(W, np.float32).T for W in (Wq, Wk, Wv)],
                        axis=1).astype(bft)
    cb = np.concatenate([
        np.eye(D, dtype=np.float32),
        np.tile(np.array([1.0], np.float32), (D, 32)),
    ], axis=1).astype(bft)
    sel = np.tile(np.array([1.0 if p % 32 == 0 else 0.0 for p in range(D)],
                  np.float32).reshape(D, 1), (1, 4))
    cf = np.concatenate([
        np.asarray(bq, np.float32).reshape(D, 1),
        sel,
        np.tile(np.asarray(bv, np.float32).reshape(1, D), (D, 4)),
    ], axis=1)
    common = {
        "w3": np.ascontiguousarray(w3),
        "cb": np.ascontiguousarray(cb),
        "cf": np.ascontiguousarray(cf),
    }
    xb = np.ascontiguousarray(input1.astype(bft))
    in_maps = [dict(common, x=xb[b]) for b in range(8)]
    res = run_bass_kernel_spmd(nc, in_maps, core_ids=list(range(8)))
    LAST_RESULTS = res
    return np.stack([r["out"].astype(np.float32) for r in res.results], axis=0)


# revision 10
# speedup vs baseline: 1.0919x; 1.0308x over previous
"""Single-head attention (B=8, S=2048, D=128) on 8 Trainium2 NeuronCores.

Sharding: data-parallel over batch - core b computes batch element b end to end
(no collectives). kernel() takes full inputs, returns the full output.

v3 design notes (per core):
  - Host casts x and [Wq.T|Wk.T|Wv.T] to bf16 (compute is bf16 anyway),
    halving input DMA and removing fp32->bf16 prologue casts. Output is
    DMA'd bf16 and widened to fp32 on host.
  - bk is dropped: softmax over keys is invariant to a per-query shift.
  - x is DMA'd shuffled (s = 16p + t; attention is permutation-equivariant,
    the output DMA inverts it) in 4 slices: x0,x2 FIFO on the sync HWDGE
    ring, x1,x3 on the scalar ring, so slices 0/1 land first and compute
    starts ~2us after the first quarter arrives.
  - PSUM: scores stage 2 slots x 2 banks + AV 1 + den 1 + prologue/epilogue
    scratch 2 = 8 banks. Prologue projections and epilogue denT/tpo flow
    through the dedicated scratch pool so they never stall the scores
    pipeline.
  - Main loop per chunk (2 k-tiles x 512 q): scoresT = kT.T @ qT (2 bf16
    matmuls N=512, fp32 psum), one ScalarE exp [128,1024] psum->sbuf bf16,
    AV accumulate (2 matmuls), den via M=32 col-packed ones matmuls every 2
    chunks. pt tiles are not reused (32 bufs) to drop a WAR sem per exp.
    Cadence is exp-bound (~1.15us).
  - Epilogue per group is split across the next group's first chunks; the
    last group takes a fast path with per-strip output DMAs.
"""

import numpy as np

S = 2048
D = 128
NT = S // 128          # 16 s-tiles of 128
NG = S // 512          # 4 q-groups of 512
NCH = 8                # chunks per group, 2 k-tiles each
SCALE = float(1.0 / np.sqrt(D))

_PROGRAM = None
LAST_RESULTS = None


def _build():
    from contextlib import ExitStack

    import concourse.bass as bass
    import concourse.mybir as mybir
    import concourse.tile as tile
    from concourse import bacc

    fp32 = mybir.dt.float32
    bf16 = mybir.dt.bfloat16
    Exp = mybir.ActivationFunctionType.Exp

    nc = bacc.Bacc(trn_type="TRN2", target_bir_lowering=False)

    x_d = nc.dram_tensor("x", [S, D], bf16, kind="ExternalInput").ap()
    w_d = nc.dram_tensor("w3", [D, 3 * D], bf16, kind="ExternalInput").ap()
    # bf16 consts: [ident(128) | ones(32) | sel(4)]
    cb_d = nc.dram_tensor("cb", [D, 164], bf16, kind="ExternalInput").ap()
    # fp32 consts: [bq(1)]; bv broadcast arrives bf16 on the slow ring
    cf_d = nc.dram_tensor("cf", [D, 1], fp32, kind="ExternalInput").ap()
    bvb_d = nc.dram_tensor("bvb", [D, 512], bf16, kind="ExternalInput").ap()
    out_d = nc.dram_tensor("out", [S, D], bf16, kind="ExternalOutput").ap()

    x_r = x_d.rearrange("(p r) d -> p r d", p=128)
    out_r = out_d.rearrange("(p r) d -> p r d", p=128)

    with tile.TileContext(nc) as tc, ExitStack() as ctx:
        singles = ctx.enter_context(tc.tile_pool(name="singles", bufs=1))
        ptp = ctx.enter_context(tc.tile_pool(name="ptp", bufs=1))
        outp = ctx.enter_context(tc.tile_pool(name="outp", bufs=1))
        # PSUM: stage 2x2 banks + av 1 + den 1 + scratch 2 = 8 banks
        stage_p = ctx.enter_context(tc.tile_pool(name="stage", bufs=1, space="PSUM"))
        av_p = ctx.enter_context(tc.tile_pool(name="av", bufs=1, space="PSUM"))
        den_p = ctx.enter_context(tc.tile_pool(name="den", bufs=1, space="PSUM"))
        pp_p = ctx.enter_context(tc.tile_pool(name="pp", bufs=2, space="PSUM"))

        # --- input DMAs. Ring order gives priority: sync ring [cb, x0, x2],
        # scalar ring [x1, x3], gpsimd (SWDGE) [w3, cf]. ---
        cb_sb = singles.tile([128, 164], bf16, tag="cb")
        nc.sync.dma_start(out=cb_sb, in_=cb_d)
        id_sb = cb_sb[:, 0:128]
        ones_sb = cb_sb[:, 128:160]
        sel_sb = cb_sb[:, 160:164]

        w3_sb = singles.tile([128, 384], bf16, tag="w3")
        nc.gpsimd.dma_start(out=w3_sb, in_=w_d)
        wq_sb = w3_sb[:, 0:128]
        wk_sb = w3_sb[:, 128:256]
        wv_sb = w3_sb[:, 256:384]

        x_sl = [None] * 4
        for h, eng in ((0, nc.sync), (1, nc.scalar), (2, nc.sync), (3, nc.scalar)):
            xh = singles.tile([128, 4, 128], bf16, tag=f"xh{h}", name=f"xh_{h}")
            eng.dma_start(out=xh, in_=x_r[:, 4 * h:4 * (h + 1), :])
            x_sl[h] = xh

        cf_sb = singles.tile([128, 1], fp32, tag="cf")
        nc.gpsimd.dma_start(out=cf_sb, in_=cf_d)
        bq_sb = cf_sb[:, 0:1]
        bvb_sb = singles.tile([128, 512], bf16, tag="bvb")
        nc.gpsimd.dma_start(out=bvb_sb, in_=bvb_d)

        # --- persistent big sbuf tensors ---
        xT_sb = singles.tile([128, S], bf16, tag="xT")   # [d, s]
        qT_sb = singles.tile([128, S], bf16, tag="qT")   # [e, s]
        kT_sb = singles.tile([128, S], bf16, tag="kT")   # [e, s]
        v_sb = singles.tile([128, S], bf16, tag="v")     # 16 tiles of [s(128), d]

        def prologue_slice(s, scalar_eng):
            """Transpose x slice s, project kT/qT/v through the scratch pool.
            scalar_eng: route the kT psum->sbuf copy to ScalarE (slice 0 only,
            pre-main)."""
            sl = slice(512 * s, 512 * (s + 1))
            tpx = pp_p.tile([128, 512], bf16, tag="pp", name=f"tpx_{s}")
            for j in range(4):
                nc.tensor.matmul(
                    tpx[:, 128 * j:128 * (j + 1)], lhsT=x_sl[s][:, j, :],
                    rhs=id_sb, is_transpose=True, start=(j == 0), stop=(j == 3),
                )
            nc.vector.tensor_copy(xT_sb[:, sl], tpx)

            pk = pp_p.tile([128, 512], fp32, tag="pp", name=f"pk_{s}")
            nc.tensor.matmul(pk, lhsT=wk_sb, rhs=xT_sb[:, sl], start=True, stop=True)
            if scalar_eng:
                nc.scalar.copy(kT_sb[:, sl], pk)
            else:
                nc.vector.tensor_copy(kT_sb[:, sl], pk)

            pq = pp_p.tile([128, 512], fp32, tag="pp", name=f"pq_{s}")
            nc.tensor.matmul(pq, lhsT=wq_sb, rhs=xT_sb[:, sl], start=True, stop=True)
            nc.vector.tensor_scalar_add(qT_sb[:, sl], pq, bq_sb)

            pv = pp_p.tile([128, 512], fp32, tag="pp", name=f"pv_{s}")
            for j in range(4):
                t = 4 * s + j
                nc.tensor.matmul(
                    pv[:, 128 * j:128 * (j + 1)],
                    lhsT=xT_sb[:, 128 * t:128 * (t + 1)], rhs=wv_sb,
                    start=(j == 0), stop=(j == 3), skip_group_check=True,
                )
            nc.vector.tensor_add(v_sb[:, sl], pv, bvb_sb)

        # --- main attention loop, software-pipelined over 2-k-tile chunks ---
        avs, dens, pts, epi = {}, {}, {}, {}

        def issue_scores(g, c):
            st = stage_p.tile([128, 1024], fp32, tag=f"s{(NCH * g + c) % 2}",
                              name=f"st_{g}_{c}")
            with nc.named_scope("scores"):
                for j in range(2):
                    kt = 2 * c + j
                    nc.tensor.matmul(
                        st[:, 512 * j:512 * (j + 1)],
                        lhsT=kT_sb[:, 128 * kt:128 * (kt + 1)],
                        rhs=qT_sb[:, 512 * g:512 * (g + 1)],
                        start=True, stop=True,
                    )
            pt = ptp.tile([128, 1024], bf16, tag="pt", name=f"pt_{g}_{c}", bufs=32)
            with nc.named_scope("exp"):
                nc.scalar.activation(pt, st, Exp, scale=SCALE)
            return pt

        def issue_den_quad(g, c0):
            # quad covers chunks c0, c0+1 (k-tiles 2*c0 .. 2*c0+3): 4
            # back-to-back M=32 col-group matmuls run concurrently in the PE.
            # Allocated at the first quad so the single-buffer rotation is
            # den_g, den_{g+1}, ... (epilogue reads den via den_fs only).
            if c0 == 0:
                dens[g] = den_p.tile([128, 512], fp32, tag="den", name=f"den_{g}")
            with nc.named_scope("den"):
                for q in range(4):
                    kt = 2 * c0 + q
                    ptq, jq = pts[g, c0 + q // 2], kt % 2
                    strip = kt % 4
                    nc.tensor.matmul(
                        dens[g][32 * strip:32 * (strip + 1), :],
                        lhsT=ones_sb,
                        rhs=ptq[:, 512 * jq:512 * (jq + 1)],
                        start=(c0 == 0), stop=(c0 == NCH - 2),
                        tile_position=(0, 32 * strip),
                        skip_group_check=True,
                    )

        def issue_avden(g, c, pt):
            pts[g, c] = pt
            with nc.named_scope("av"):
                for j in range(2):
                    kt = 2 * c + j
                    nc.tensor.matmul(
                        avs[g], lhsT=v_sb[:, 128 * kt:128 * (kt + 1)],
                        rhs=pt[:, 512 * j:512 * (j + 1)],
                        start=(kt == 0), stop=(kt == 15),
                    )
            if c % 2 == 0 and c > 0:
                issue_den_quad(g, c - 2)

        def epilogue_a(g):
            """av copy (frees the av bank for g+1) + den copy (frees den)."""
            av, den = avs.pop(g), dens.pop(g)
            with nc.named_scope("epi"):
                oT_sb = outp.tile([128, 512], bf16, tag="oTsb", name=f"oTsb_{g}",
                                  bufs=2)
                den_fs = outp.tile([128, 512], bf16, tag="denfs", name=f"denfs_{g}",
                                   bufs=2)
                nc.vector.tensor_copy(oT_sb, av)
                nc.vector.tensor_copy(den_fs, den)
            epi[g] = (oT_sb, den_fs)

        def epilogue_b(g):
            """selector matmuls -> reciprocal (den path, through scratch)."""
            oT_sb, den_fs = epi[g]
            with nc.named_scope("epi"):
                denT = pp_p.tile([128, 16], fp32, tag="pp", name=f"denT_{g}")
                for j in range(4):
                    nc.tensor.matmul(
                        denT[:, 4 * j:4 * (j + 1)],
                        lhsT=den_fs[:, 128 * j:128 * (j + 1)],
                        rhs=sel_sb, start=(j == 0), stop=(j == 3),
                    )
                recip = outp.tile([128, 16], fp32, tag="recip", name=f"recip_{g}",
                                  bufs=2)
                nc.vector.reciprocal(recip, denT)
            epi[g] = (oT_sb, recip)

        def epilogue_c(g, split_dma=False):
            """transpose back to [q, d], scale by 1/den, DMA out."""
            oT_sb, recip = epi.pop(g)
            with nc.named_scope("epi"):
                tpo = pp_p.tile([128, 512], bf16, tag="pp", name=f"tpo_{g}")
                for j in range(4):
                    nc.tensor.matmul(
                        tpo[:, 128 * j:128 * (j + 1)],
                        lhsT=oT_sb[:, 128 * j:128 * (j + 1)], rhs=id_sb,
                        is_transpose=True, start=(j == 0), stop=(j == 3),
                    )
                osb = outp.tile([128, 512], bf16, tag="osb", name=f"osb_{g}", bufs=2)
                osb_r = osb.rearrange("p (j d) -> p j d", j=4)
                if split_dma:
                    for j in range(4):
                        nc.vector.tensor_scalar_mul(
                            osb[:, 128 * j:128 * (j + 1)],
                            tpo[:, 128 * j:128 * (j + 1)],
                            recip[:, 4 * j:4 * j + 1],
                        )
                        eng = nc.sync if j % 2 == 0 else nc.scalar
                        eng.dma_start(
                            out=out_r[:, 4 * g + j:4 * g + j + 1, :],
                            in_=osb_r[:, j:j + 1, :],
                        )
                else:
                    for j in range(4):
                        nc.vector.tensor_scalar_mul(
                            osb[:, 128 * j:128 * (j + 1)],
                            tpo[:, 128 * j:128 * (j + 1)],
                            recip[:, 4 * j:4 * j + 1],
                        )
                    nc.sync.dma_start(
                        out=out_r[:, 4 * g:4 * (g + 1), :], in_=osb_r,
                    )

        prologue_slice(0, scalar_eng=True)
        prologue_slice(1, scalar_eng=False)
        prologue_at = {(0, 2): 2, (0, 4): 3}

        chunks = [(g, c) for g in range(NG) for c in range(NCH)]
        prev = None
        for g, c in chunks:
            if (g, c) in prologue_at:
                prologue_slice(prologue_at[(g, c)], scalar_eng=False)
            if c == 0:
                avs[g] = av_p.tile([128, 512], fp32, tag="av", name=f"av_{g}")
            pt = issue_scores(g, c)
            if prev is not None:
                issue_avden(*prev)
                if prev[1] == NCH - 1:
                    issue_den_quad(prev[0], NCH - 2)
                    epilogue_a(prev[0])
            if c == 1 and g > 0:
                epilogue_b(g - 1)
            if c == 2 and g > 0:
                epilogue_c(g - 1)
            prev = (g, c, pt)
        issue_avden(*prev)
        issue_den_quad(prev[0], NCH - 2)
        epilogue_a(NG - 1)
        epilogue_b(NG - 1)
        epilogue_c(NG - 1, split_dma=True)

    nc.compile()
    return nc


def _get_program():
    global _PROGRAM
    if _PROGRAM is None:
        _PROGRAM = _build()
    return _PROGRAM


def _ensure_axon_hooks():
    """bass_utils imports antenv.axon_hooks when tracing; provide a stub if
    the image's antenv lacks it (hook defaults to None => tracing skipped)."""
    import sys
    import types
    try:
        import antenv.axon_hooks  # noqa: F401
        return
    except ImportError:
        pass
    import antenv
    m = types.ModuleType("antenv.axon_hooks")
    m._hook = None
    def _set(h):
        m._hook = h
    def _get():
        return m._hook
    m.set_axon_ntff_profile_hook = _set
    m.get_axon_ntff_profile_hook = _get
    sys.modules["antenv.axon_hooks"] = m
    antenv.axon_hooks = m


def kernel(input1, Wq, bq, Wk, bk, Wv, bv):
    global LAST_RESULTS
    _ensure_axon_hooks()
    import ml_dtypes
    from concourse.bass_utils import run_bass_kernel_spmd

    nc = _get_program()
    bft = ml_dtypes.bfloat16

    input1 = np.asarray(input1, dtype=np.float32)
    w3 = np.concatenate([np.asarray(W, np.float32).T for W in (Wq, Wk, Wv)],
                        axis=1).astype(bft)
    sel = np.tile(np.array([1.0 if p % 32 == 0 else 0.0 for p in range(D)],
                  np.float32).reshape(D, 1), (1, 4))
    cb = np.concatenate([
        np.eye(D, dtype=np.float32),
        np.ones((D, 32), np.float32),
        sel,
    ], axis=1).astype(bft)
    cf = np.asarray(bq, np.float32).reshape(D, 1)
    bvb = np.tile(np.asarray(bv, np.float32).reshape(1, D), (D, 4)).astype(bft)
    common = {
        "w3": np.ascontiguousarray(w3),
        "cb": np.ascontiguousarray(cb),
        "cf": np.ascontiguousarray(cf),
        "bvb": np.ascontiguousarray(bvb),
    }
    xb = np.ascontiguousarray(input1.astype(bft))
    in_maps = [dict(common, x=xb[b]) for b in range(8)]
    res = run_bass_kernel_spmd(nc, in_maps, core_ids=list(range(8)))
    LAST_RESULTS = res
    return np.stack([r["out"].astype(np.float32) for r in res.results], axis=0)


# revision 16
# speedup vs baseline: 1.1835x; 1.0839x over previous
"""Single-head attention (B=8, S=2048, D=128) on 8 Trainium2 NeuronCores.

Sharding: data-parallel over batch - core b computes batch element b end to end
(no collectives). kernel() takes full inputs, returns the full output.

v3 design notes (per core):
  - Host casts x and [Wq.T|Wk.T|Wv.T] to bf16 (compute is bf16 anyway),
    halving input DMA and removing fp32->bf16 prologue casts. Output is
    DMA'd bf16 and widened to fp32 on host.
  - bk is dropped: softmax over keys is invariant to a per-query shift.
  - x is DMA'd shuffled (s = 16p + t; attention is permutation-equivariant,
    the output DMA inverts it) in 4 slices: x0,x2 FIFO on the sync HWDGE
    ring, x1,x3 on the scalar ring, so slices 0/1 land first and compute
    starts ~2us after the first quarter arrives.
  - PSUM: scores stage 2 slots x 2 banks + AV 1 + den 1 + prologue/epilogue
    scratch 2 = 8 banks. Prologue projections and epilogue denT/tpo flow
    through the dedicated scratch pool so they never stall the scores
    pipeline.
  - Main loop per chunk (2 k-tiles x 512 q): scoresT = kT.T @ qT (2 bf16
    matmuls N=512, fp32 psum), one ScalarE exp [128,1024] psum->sbuf bf16,
    AV accumulate (2 matmuls), den via M=32 col-packed ones matmuls every 2
    chunks. pt tiles are not reused (32 bufs) to drop a WAR sem per exp.
    Cadence is exp-bound (~1.15us).
  - Epilogue per group is split across the next group's first chunks; the
    last group takes a fast path with per-strip output DMAs.
"""

import numpy as np

S = 2048
D = 128
NT = S // 128          # 16 s-tiles of 128
NG = S // 512          # 4 q-groups of 512
NCH = 8                # chunks per group, 2 k-tiles each
SCALE = float(1.0 / np.sqrt(D))

_PROGRAM = None
LAST_RESULTS = None


def _build():
    from contextlib import ExitStack

    import concourse.bass as bass
    import concourse.mybir as mybir
    import concourse.tile as tile
    from concourse import bacc

    fp32 = mybir.dt.float32
    bf16 = mybir.dt.bfloat16
    Exp = mybir.ActivationFunctionType.Exp

    nc = bacc.Bacc(trn_type="TRN2", target_bir_lowering=False)

    x_d = nc.dram_tensor("x", [S, D], bf16, kind="ExternalInput").ap()
    w_d = nc.dram_tensor("w3", [D, 3 * D], bf16, kind="ExternalInput").ap()
    # bf16 consts: [ident(128) | ones(32) | sel(4)]
    cb_d = nc.dram_tensor("cb", [D, 164], bf16, kind="ExternalInput").ap()
    # fp32 consts: [bq(1)]; bv broadcast arrives bf16 on the slow ring
    cf_d = nc.dram_tensor("cf", [D, 1], fp32, kind="ExternalInput").ap()
    bvb_d = nc.dram_tensor("bvb", [D, 512], bf16, kind="ExternalInput").ap()
    out_d = nc.dram_tensor("out", [S, D], bf16, kind="ExternalOutput").ap()

    x_r = x_d.rearrange("(p r) d -> p r d", p=128)
    out_r = out_d.rearrange("(p r) d -> p r d", p=128)

    with tile.TileContext(nc) as tc, ExitStack() as ctx:
        singles = ctx.enter_context(tc.tile_pool(name="singles", bufs=1))
        ptp = ctx.enter_context(tc.tile_pool(name="ptp", bufs=1))
        outp = ctx.enter_context(tc.tile_pool(name="outp", bufs=1))
        # PSUM: stage 2x2 banks + av 1 + den 1 + scratch 2 = 8 banks
        stage_p = ctx.enter_context(tc.tile_pool(name="stage", bufs=1, space="PSUM"))
        av_p = ctx.enter_context(tc.tile_pool(name="av", bufs=1, space="PSUM"))
        den_p = ctx.enter_context(tc.tile_pool(name="den", bufs=1, space="PSUM"))
        pp_p = ctx.enter_context(tc.tile_pool(name="pp", bufs=2, space="PSUM"))

        # --- input DMAs. Ring order gives priority: sync ring [x0, x2],
        # scalar ring [cb, x1, x3], gpsimd (SWDGE) [w3, cf, bvb]. ---
        cb_sb = singles.tile([128, 164], bf16, tag="cb")
        nc.scalar.dma_start(out=cb_sb, in_=cb_d)
        id_sb = cb_sb[:, 0:128]
        ones_sb = cb_sb[:, 128:160]
        sel_sb = cb_sb[:, 160:164]

        w3_sb = singles.tile([128, 384], bf16, tag="w3")
        nc.gpsimd.dma_start(out=w3_sb, in_=w_d)
        wq_sb = w3_sb[:, 0:128]
        wk_sb = w3_sb[:, 128:256]
        wv_sb = w3_sb[:, 256:384]

        x_sl = [None] * 4
        for h, eng in ((0, nc.sync), (1, nc.scalar), (2, nc.sync), (3, nc.scalar)):
            xh = singles.tile([128, 4, 128], bf16, tag=f"xh{h}", name=f"xh_{h}")
            eng.dma_start(out=xh, in_=x_r[:, 4 * h:4 * (h + 1), :])
            x_sl[h] = xh

        cf_sb = singles.tile([128, 1], fp32, tag="cf")
        nc.gpsimd.dma_start(out=cf_sb, in_=cf_d)
        bq_sb = cf_sb[:, 0:1]
        bvb_sb = singles.tile([128, 512], bf16, tag="bvb")
        nc.gpsimd.dma_start(out=bvb_sb, in_=bvb_d)

        # --- persistent big sbuf tensors ---
        xT_sb = singles.tile([128, S], bf16, tag="xT")   # [d, s]
        qT_sb = singles.tile([128, S], bf16, tag="qT")   # [e, s]
        kT_sb = singles.tile([128, S], bf16, tag="kT")   # [e, s]
        v_sb = singles.tile([128, S], bf16, tag="v")     # 16 tiles of [s(128), d]

        def prologue_kT(s, scalar_eng=False):
            """Transpose x slice s and project kT (the part that gates the
            scores pipeline). scalar_eng routes the kT psum->sbuf copy to
            ScalarE (slice 0 only, pre-main)."""
            sl = slice(512 * s, 512 * (s + 1))
            tpx = pp_p.tile([128, 512], bf16, tag="pp", name=f"tpx_{s}")
            for j in range(4):
                nc.tensor.matmul(
                    tpx[:, 128 * j:128 * (j + 1)], lhsT=x_sl[s][:, j, :],
                    rhs=id_sb, is_transpose=True, start=(j == 0), stop=(j == 3),
                )
            nc.vector.tensor_copy(xT_sb[:, sl], tpx)

            pk = pp_p.tile([128, 512], fp32, tag="pp", name=f"pk_{s}")
            nc.tensor.matmul(pk, lhsT=wk_sb, rhs=xT_sb[:, sl], start=True, stop=True)
            if scalar_eng:
                nc.scalar.copy(kT_sb[:, sl], pk)
            else:
                nc.vector.tensor_copy(kT_sb[:, sl], pk)

        def prologue_qv(s):
            """Project qT (needed only from group s) and v (lags AV by 2+
            chunks) for slice s."""
            sl = slice(512 * s, 512 * (s + 1))
            pq = pp_p.tile([128, 512], fp32, tag="pp", name=f"pq_{s}")
            nc.tensor.matmul(pq, lhsT=wq_sb, rhs=xT_sb[:, sl], start=True, stop=True)
            nc.vector.tensor_scalar_add(qT_sb[:, sl], pq, bq_sb)

            pv = pp_p.tile([128, 512], fp32, tag="pp", name=f"pv_{s}")
            for j in range(4):
                t = 4 * s + j
                nc.tensor.matmul(
                    pv[:, 128 * j:128 * (j + 1)],
                    lhsT=xT_sb[:, 128 * t:128 * (t + 1)], rhs=wv_sb,
                    start=(j == 0), stop=(j == 3), skip_group_check=True,
                )
            nc.vector.tensor_add(v_sb[:, sl], pv, bvb_sb)

        # --- main attention loop, software-pipelined over 2-k-tile chunks.
        # AV runs at lag 2 behind scores/exp so the exp-done semaphore each AV
        # waits on is long settled when the PE reaches it (no per-chunk stall).
        avs, dens, pts, epi = {}, {}, {}, {}

        def issue_scores(g, c):
            st = stage_p.tile([128, 1024], fp32, tag=f"s{(NCH * g + c) % 2}",
                              name=f"st_{g}_{c}")
            with nc.named_scope("scores"):
                for j in range(2):
                    kt = 2 * c + j
                    nc.tensor.matmul(
                        st[:, 512 * j:512 * (j + 1)],
                        lhsT=kT_sb[:, 128 * kt:128 * (kt + 1)],
                        rhs=qT_sb[:, 512 * g:512 * (g + 1)],
                        start=True, stop=True,
                    )
            pt = ptp.tile([128, 1024], bf16, tag="pt", name=f"pt_{g}_{c}", bufs=32)
            with nc.named_scope("exp"):
                nc.scalar.activation(pt, st, Exp, scale=SCALE)
            return pt

        def issue_den_quad(g, c0):
            # quad covers chunks c0, c0+1 (k-tiles 2*c0 .. 2*c0+3): 4
            # back-to-back M=32 col-group matmuls run concurrently in the PE.
            # Allocated at the first quad so the single-buffer rotation is
            # den_g, den_{g+1}, ... (epilogue reads den via den_fs only).
            if c0 == 0:
                dens[g] = den_p.tile([128, 512], fp32, tag="den", name=f"den_{g}")
            with nc.named_scope("den"):
                for q in range(4):
                    kt = 2 * c0 + q
                    ptq, jq = pts[g, c0 + q // 2], kt % 2
                    strip = kt % 4
                    nc.tensor.matmul(
                        dens[g][32 * strip:32 * (strip + 1), :],
                        lhsT=ones_sb,
                        rhs=ptq[:, 512 * jq:512 * (jq + 1)],
                        start=(c0 == 0), stop=(c0 == NCH - 2),
                        tile_position=(0, 32 * strip),
                        skip_group_check=True,
                    )

        def issue_avden(g, c, pt):
            pts[g, c] = pt
            with nc.named_scope("av"):
                for j in range(2):
                    kt = 2 * c + j
                    nc.tensor.matmul(
                        avs[g], lhsT=v_sb[:, 128 * kt:128 * (kt + 1)],
                        rhs=pt[:, 512 * j:512 * (j + 1)],
                        start=(kt == 0), stop=(kt == 15),
                    )
            if c % 2 == 0 and c > 0:
                issue_den_quad(g, c - 2)

        def epilogue_a(g, last=False):
            """av copy (frees the av bank for g+1) + den copy (frees den)."""
            av, den = avs.pop(g), dens.pop(g)
            with nc.named_scope("epi"):
                oT_sb = outp.tile([128, 512], bf16, tag="oTsb", name=f"oTsb_{g}",
                                  bufs=2)
                den_fs = outp.tile([128, 512], bf16, tag="denfs", name=f"denfs_{g}",
                                   bufs=2)
                if last:
                    # den path first (it gates recip), av in per-strip pieces
                    # so each output strip's transpose can start asap.
                    nc.vector.tensor_copy(den_fs, den)
                    for j in range(4):
                        nc.vector.tensor_copy(
                            oT_sb[:, 128 * j:128 * (j + 1)],
                            av[:, 128 * j:128 * (j + 1)],
                        )
                else:
                    nc.vector.tensor_copy(oT_sb, av)
                    nc.vector.tensor_copy(den_fs, den)
            epi[g] = (oT_sb, den_fs)

        def epilogue_b(g):
            """selector matmuls -> reciprocal (den path, through scratch)."""
            oT_sb, den_fs = epi[g]
            with nc.named_scope("epi"):
                denT = pp_p.tile([128, 16], fp32, tag="pp", name=f"denT_{g}")
                for j in range(4):
                    nc.tensor.matmul(
                        denT[:, 4 * j:4 * (j + 1)],
                        lhsT=den_fs[:, 128 * j:128 * (j + 1)],
                        rhs=sel_sb, start=(j == 0), stop=(j == 3),
                    )
                recip = outp.tile([128, 16], fp32, tag="recip", name=f"recip_{g}",
                                  bufs=2)
                nc.vector.reciprocal(recip, denT)
            epi[g] = (oT_sb, recip)

        def epilogue_c(g, split_dma=False):
            """transpose back to [q, d], scale by 1/den, DMA out."""
            oT_sb, recip = epi.pop(g)
            with nc.named_scope("epi"):
                tpo = pp_p.tile([128, 512], bf16, tag="pp", name=f"tpo_{g}")
                for j in range(4):
                    nc.tensor.matmul(
                        tpo[:, 128 * j:128 * (j + 1)],
                        lhsT=oT_sb[:, 128 * j:128 * (j + 1)], rhs=id_sb,
                        is_transpose=True, start=(j == 0), stop=(j == 3),
                    )
                osb = outp.tile([128, 512], bf16, tag="osb", name=f"osb_{g}", bufs=2)
                osb_r = osb.rearrange("p (j d) -> p j d", j=4)
                if split_dma:
                    for j in range(4):
                        nc.vector.tensor_scalar_mul(
                            osb[:, 128 * j:128 * (j + 1)],
                            tpo[:, 128 * j:128 * (j + 1)],
                            recip[:, 4 * j:4 * j + 1],
                        )
                        eng = nc.sync if j % 2 == 0 else nc.scalar
                        eng.dma_start(
                            out=out_r[:, 4 * g + j:4 * g + j + 1, :],
                            in_=osb_r[:, j:j + 1, :],
                        )
                else:
                    for j in range(4):
                        nc.vector.tensor_scalar_mul(
                            osb[:, 128 * j:128 * (j + 1)],
                            tpo[:, 128 * j:128 * (j + 1)],
                            recip[:, 4 * j:4 * j + 1],
                        )
                    nc.sync.dma_start(
                        out=out_r[:, 4 * g:4 * (g + 1), :], in_=osb_r,
                    )

        prologue_kT(0, scalar_eng=True)
        prologue_qv(0)
        prologue_at = {
            (0, 1): lambda: prologue_kT(1), (0, 2): lambda: prologue_qv(1),
            (0, 3): lambda: prologue_kT(2), (0, 4): lambda: prologue_qv(2),
            (0, 5): lambda: prologue_kT(3), (0, 6): lambda: prologue_qv(3),
        }

        chunks = [(g, c) for g in range(NG) for c in range(NCH)]
        lag = [None, None]  # 2-deep AV lag queue
        for g, c in chunks:
            if (g, c) in prologue_at:
                prologue_at[(g, c)]()
            if c == 0:
                avs[g] = av_p.tile([128, 512], fp32, tag="av", name=f"av_{g}")
            pt = issue_scores(g, c)
            if lag[0] is not None:
                issue_avden(*lag[0])
                if lag[0][1] == NCH - 1:
                    gp = lag[0][0]
                    issue_den_quad(gp, NCH - 2)
                    epilogue_a(gp)
            lag = [lag[1], (g, c, pt)]
            if c == 4 and g > 0:
                epilogue_b(g - 1)
            if c == 6 and g > 0:
                epilogue_c(g - 1)
        # drain: AV for the last two chunks, final den quad, fast epilogue
        for item in lag:
            issue_avden(*item)
        issue_den_quad(NG - 1, NCH - 2)
        epilogue_a(NG - 1, last=True)
        epilogue_b(NG - 1)
        epilogue_c(NG - 1, split_dma=True)

    nc.compile()
    return nc


def _get_program():
    global _PROGRAM
    if _PROGRAM is None:
        _PROGRAM = _build()
    return _PROGRAM


def _ensure_axon_hooks():
    """bass_utils imports antenv.axon_hooks when tracing; provide a stub if
    the image's antenv lacks it (hook defaults to None => tracing skipped)."""
    import sys
    import types
    try:
        import antenv.axon_hooks  # noqa: F401
        return
    except ImportError:
        pass
    import antenv
    m = types.ModuleType("antenv.axon_hooks")
    m._hook = None
    def _set(h):
        m._hook = h
    def _get():
        return m._hook
    m.set_axon_ntff_profile_hook = _set
    m.get_axon_ntff_profile_hook = _get
    sys.modules["antenv.axon_hooks"] = m
    antenv.axon_hooks = m


def kernel(input1, Wq, bq, Wk, bk, Wv, bv):
    global LAST_RESULTS
    _ensure_axon_hooks()
    import ml_dtypes
    from concourse.bass_utils import run_bass_kernel_spmd

    nc = _get_program()
    bft = ml_dtypes.bfloat16

    input1 = np.asarray(input1, dtype=np.float32)
    w3 = np.concatenate([np.asarray(W, np.float32).T for W in (Wq, Wk, Wv)],
                        axis=1).astype(bft)
    sel = np.tile(np.array([1.0 if p % 32 == 0 else 0.0 for p in range(D)],
                  np.float32).reshape(D, 1), (1, 4))
    cb = np.concatenate([
        np.eye(D, dtype=np.float32),
        np.ones((D, 32), np.float32),
        sel,
    ], axis=1).astype(bft)
    cf = np.asarray(bq, np.float32).reshape(D, 1)
    bvb = np.tile(np.asarray(bv, np.float32).reshape(1, D), (D, 4)).astype(bft)
    common = {
        "w3": np.ascontiguousarray(w3),
        "cb": np.ascontiguousarray(cb),
        "cf": np.ascontiguousarray(cf),
        "bvb": np.ascontiguousarray(bvb),
    }
    xb = np.ascontiguousarray(input1.astype(bft))
    in_maps = [dict(common, x=xb[b]) for b in range(8)]
    res = run_bass_kernel_spmd(nc, in_maps, core_ids=list(range(8)))
    LAST_RESULTS = res
    return np.stack([r["out"].astype(np.float32) for r in res.results], axis=0)


# revision 22
# speedup vs baseline: 1.2006x; 1.0145x over previous
"""Single-head attention (B=8, S=2048, D=128) on 8 Trainium2 NeuronCores.

Sharding: data-parallel over batch - core b computes batch element b end to end
(no collectives). kernel() takes full inputs, returns the full output.

v3 design notes (per core):
  - Host casts x and [Wq.T|Wk.T|Wv.T] to bf16 (compute is bf16 anyway),
    halving input DMA and removing fp32->bf16 prologue casts. Output is
    DMA'd bf16 and widened to fp32 on host.
  - bk is dropped: softmax over keys is invariant to a per-query shift.
  - x is DMA'd shuffled (s = 16p + t; attention is permutation-equivariant,
    the output DMA inverts it) in 4 slices: x0,x2 FIFO on the sync HWDGE
    ring, x1,x3 on the scalar ring, so slices 0/1 land first and compute
    starts ~2us after the first quarter arrives.
  - PSUM: scores stage 2 slots x 2 banks + AV 1 + den 1 + prologue/epilogue
    scratch 2 = 8 banks. Prologue projections and epilogue denT/tpo flow
    through the dedicated scratch pool so they never stall the scores
    pipeline.
  - Main loop per chunk (2 k-tiles x 512 q): scoresT = kT.T @ qT (2 bf16
    matmuls N=512, fp32 psum), one ScalarE exp [128,1024] psum->sbuf bf16,
    AV accumulate (2 matmuls), den via M=32 col-packed ones matmuls every 2
    chunks. pt tiles are not reused (32 bufs) to drop a WAR sem per exp.
    Cadence is exp-bound (~1.15us).
  - Epilogue per group is split across the next group's first chunks; the
    last group takes a fast path with per-strip output DMAs.
"""

import numpy as np

S = 2048
D = 128
NT = S // 128          # 16 s-tiles of 128
NG = S // 512          # 4 q-groups of 512
NCH = 8                # chunks per group, 2 k-tiles each
SCALE = float(1.0 / np.sqrt(D))

_PROGRAM = None
LAST_RESULTS = None


def _build():
    from contextlib import ExitStack

    import concourse.bass as bass
    import concourse.mybir as mybir
    import concourse.tile as tile
    from concourse import bacc

    fp32 = mybir.dt.float32
    bf16 = mybir.dt.bfloat16
    Exp = mybir.ActivationFunctionType.Exp

    nc = bacc.Bacc(trn_type="TRN2", target_bir_lowering=False)

    x_d = nc.dram_tensor("x", [S, D], bf16, kind="ExternalInput").ap()
    w_d = nc.dram_tensor("w3", [D, 3 * D], bf16, kind="ExternalInput").ap()
    # bf16 consts: [ident(128) | ones(32) | sel(4)]
    cb_d = nc.dram_tensor("cb", [D, 164], bf16, kind="ExternalInput").ap()
    # fp32 consts: [bq(1)]; bv broadcast arrives bf16 on the slow ring
    cf_d = nc.dram_tensor("cf", [D, 1], fp32, kind="ExternalInput").ap()
    bvb_d = nc.dram_tensor("bvb", [D, 512], bf16, kind="ExternalInput").ap()
    out_d = nc.dram_tensor("out", [S, D], bf16, kind="ExternalOutput").ap()

    x_r = x_d.rearrange("(p r) d -> p r d", p=128)
    out_r = out_d.rearrange("(p r) d -> p r d", p=128)

    with tile.TileContext(nc) as tc, ExitStack() as ctx:
        singles = ctx.enter_context(tc.tile_pool(name="singles", bufs=1))
        ptp = ctx.enter_context(tc.tile_pool(name="ptp", bufs=1))
        outp = ctx.enter_context(tc.tile_pool(name="outp", bufs=1))
        # PSUM: stage 2x2 banks + av 1 + den 1 + scratch 2 = 8 banks
        stage_p = ctx.enter_context(tc.tile_pool(name="stage", bufs=1, space="PSUM"))
        av_p = ctx.enter_context(tc.tile_pool(name="av", bufs=1, space="PSUM"))
        den_p = ctx.enter_context(tc.tile_pool(name="den", bufs=1, space="PSUM"))
        pp_p = ctx.enter_context(tc.tile_pool(name="pp", bufs=2, space="PSUM"))

        # --- input DMAs. Ring order gives priority: sync ring [x0, x2],
        # scalar ring [cb, x1, x3], gpsimd (SWDGE) [w3, cf, bvb]. ---
        cb_sb = singles.tile([128, 164], bf16, tag="cb")
        nc.scalar.dma_start(out=cb_sb, in_=cb_d)
        id_sb = cb_sb[:, 0:128]
        ones_sb = cb_sb[:, 128:160]
        sel_sb = cb_sb[:, 160:164]

        w3_sb = singles.tile([128, 384], bf16, tag="w3")
        nc.gpsimd.dma_start(out=w3_sb, in_=w_d)
        wq_sb = w3_sb[:, 0:128]
        wk_sb = w3_sb[:, 128:256]
        wv_sb = w3_sb[:, 256:384]

        x_sl = [None] * 4
        for h, eng in ((0, nc.sync), (1, nc.scalar), (2, nc.sync), (3, nc.scalar)):
            xh = singles.tile([128, 4, 128], bf16, tag=f"xh{h}", name=f"xh_{h}")
            eng.dma_start(out=xh, in_=x_r[:, 4 * h:4 * (h + 1), :])
            x_sl[h] = xh

        cf_sb = singles.tile([128, 1], fp32, tag="cf")
        nc.gpsimd.dma_start(out=cf_sb, in_=cf_d)
        bq_sb = cf_sb[:, 0:1]
        bvb_sb = singles.tile([128, 512], bf16, tag="bvb")
        nc.gpsimd.dma_start(out=bvb_sb, in_=bvb_d)

        # --- persistent big sbuf tensors ---
        xT_sb = singles.tile([128, S], bf16, tag="xT")   # [d, s]
        qT_sb = singles.tile([128, S], bf16, tag="qT")   # [e, s]
        kT_sb = singles.tile([128, S], bf16, tag="kT")   # [e, s]
        v_sb = singles.tile([128, S], bf16, tag="v")     # 16 tiles of [s(128), d]

        # --- PE warm-up: the HAM clock gate keeps the PE at 1.2 GHz until it
        # sees ~3.4us of sustained activity. The PE is idle waiting for the x
        # DMA anyway, so burn junk matmuls (gated only on a local memset) to
        # reach 2.4 GHz before the first real transpose. ---
        junk = singles.tile([128, 128], bf16, tag="junk")
        nc.vector.memset(junk, 1.0)
        warm = pp_p.tile([128, 512], fp32, tag="pp", name="warm")
        for i in range(30):
            nc.tensor.matmul(warm[:, 0:128], lhsT=junk, rhs=junk,
                             start=True, stop=True, skip_group_check=True)
        warm_rd = singles.tile([128, 1], fp32, tag="warmrd")
        nc.vector.tensor_copy(warm_rd, warm[:, 0:1])

        def prologue_kT(s, scalar_eng=False):
            """Transpose x slice s and project kT (the part that gates the
            scores pipeline). scalar_eng routes the kT psum->sbuf copy to
            ScalarE (slice 0 only, pre-main)."""
            sl = slice(512 * s, 512 * (s + 1))
            tpx = pp_p.tile([128, 512], bf16, tag="pp", name=f"tpx_{s}")
            for j in range(4):
                nc.tensor.matmul(
                    tpx[:, 128 * j:128 * (j + 1)], lhsT=x_sl[s][:, j, :],
                    rhs=id_sb, is_transpose=True, start=(j == 0), stop=(j == 3),
                )
            nc.vector.tensor_copy(xT_sb[:, sl], tpx)

            pk = pp_p.tile([128, 512], fp32, tag="pp", name=f"pk_{s}")
            nc.tensor.matmul(pk, lhsT=wk_sb, rhs=xT_sb[:, sl], start=True, stop=True)
            if scalar_eng:
                nc.scalar.copy(kT_sb[:, sl], pk)
            else:
                nc.vector.tensor_copy(kT_sb[:, sl], pk)

        def prologue_q(s):
            """Project qT for slice s (needed only from group s)."""
            sl = slice(512 * s, 512 * (s + 1))
            pq = pp_p.tile([128, 512], fp32, tag="pp", name=f"pq_{s}")
            nc.tensor.matmul(pq, lhsT=wq_sb, rhs=xT_sb[:, sl], start=True, stop=True)
            nc.vector.tensor_scalar_add(qT_sb[:, sl], pq, bq_sb)

        def prologue_v(s):
            """Project v for slice s (first used by AV of chunk 2s, which runs
            at lag 2 behind scores)."""
            sl = slice(512 * s, 512 * (s + 1))
            pv = pp_p.tile([128, 512], fp32, tag="pp", name=f"pv_{s}")
            for j in range(4):
                t = 4 * s + j
                nc.tensor.matmul(
                    pv[:, 128 * j:128 * (j + 1)],
                    lhsT=xT_sb[:, 128 * t:128 * (t + 1)], rhs=wv_sb,
                    start=(j == 0), stop=(j == 3), skip_group_check=True,
                )
            nc.vector.tensor_add(v_sb[:, sl], pv, bvb_sb)

        # --- main attention loop, software-pipelined over 2-k-tile chunks.
        # AV runs at lag 2 behind scores/exp so the exp-done semaphore each AV
        # waits on is long settled when the PE reaches it (no per-chunk stall).
        avs, dens, pts, epi = {}, {}, {}, {}

        def issue_scores(g, c):
            st = stage_p.tile([128, 1024], fp32, tag=f"s{(NCH * g + c) % 2}",
                              name=f"st_{g}_{c}")
            with nc.named_scope("scores"):
                for j in range(2):
                    kt = 2 * c + j
                    nc.tensor.matmul(
                        st[:, 512 * j:512 * (j + 1)],
                        lhsT=kT_sb[:, 128 * kt:128 * (kt + 1)],
                        rhs=qT_sb[:, 512 * g:512 * (g + 1)],
                        start=True, stop=True,
                    )
            pt = ptp.tile([128, 1024], bf16, tag="pt", name=f"pt_{g}_{c}", bufs=32)
            with nc.named_scope("exp"):
                nc.scalar.activation(pt, st, Exp, scale=SCALE)
            return pt

        def issue_den_quad(g, c0):
            # quad covers chunks c0, c0+1 (k-tiles 2*c0 .. 2*c0+3): 4
            # back-to-back M=32 col-group matmuls run concurrently in the PE.
            # Allocated at the first quad so the single-buffer rotation is
            # den_g, den_{g+1}, ... (epilogue reads den via den_fs only).
            if c0 == 0:
                dens[g] = den_p.tile([128, 512], fp32, tag="den", name=f"den_{g}")
            with nc.named_scope("den"):
                for q in range(4):
                    kt = 2 * c0 + q
                    ptq, jq = pts[g, c0 + q // 2], kt % 2
                    strip = kt % 4
                    nc.tensor.matmul(
                        dens[g][32 * strip:32 * (strip + 1), :],
                        lhsT=ones_sb,
                        rhs=ptq[:, 512 * jq:512 * (jq + 1)],
                        start=(c0 == 0), stop=(c0 == NCH - 2),
                        tile_position=(0, 32 * strip),
                        skip_group_check=True,
                    )

        def issue_avden(g, c, pt):
            pts[g, c] = pt
            with nc.named_scope("av"):
                for j in range(2):
                    kt = 2 * c + j
                    nc.tensor.matmul(
                        avs[g], lhsT=v_sb[:, 128 * kt:128 * (kt + 1)],
                        rhs=pt[:, 512 * j:512 * (j + 1)],
                        start=(kt == 0), stop=(kt == 15),
                    )
            if c % 2 == 0 and c > 0:
                issue_den_quad(g, c - 2)

        def epilogue_a(g, last=False):
            """av copy (frees the av bank for g+1) + den copy (frees den)."""
            av, den = avs.pop(g), dens.pop(g)
            with nc.named_scope("epi"):
                oT_sb = outp.tile([128, 512], bf16, tag="oTsb", name=f"oTsb_{g}",
                                  bufs=2)
                den_fs = outp.tile([128, 512], bf16, tag="denfs", name=f"denfs_{g}",
                                   bufs=2)
                if last:
                    # den path first: it gates the reciprocal chain.
                    nc.vector.tensor_copy(den_fs, den)
                    nc.vector.tensor_copy(oT_sb, av)
                else:
                    nc.vector.tensor_copy(oT_sb, av)
                    nc.vector.tensor_copy(den_fs, den)
            epi[g] = (oT_sb, den_fs)

        def epilogue_b(g):
            """selector matmuls -> reciprocal (den path, through scratch)."""
            oT_sb, den_fs = epi[g]
            with nc.named_scope("epi"):
                denT = pp_p.tile([128, 16], fp32, tag="pp", name=f"denT_{g}")
                for j in range(4):
                    nc.tensor.matmul(
                        denT[:, 4 * j:4 * (j + 1)],
                        lhsT=den_fs[:, 128 * j:128 * (j + 1)],
                        rhs=sel_sb, start=(j == 0), stop=(j == 3),
                    )
                recip = outp.tile([128, 16], fp32, tag="recip", name=f"recip_{g}",
                                  bufs=2)
                nc.vector.reciprocal(recip, denT)
            epi[g] = (oT_sb, recip)

        def epilogue_c(g, split_dma=False):
            """transpose back to [q, d], scale by 1/den, DMA out."""
            oT_sb, recip = epi.pop(g)
            with nc.named_scope("epi"):
                tpo = pp_p.tile([128, 512], bf16, tag="pp", name=f"tpo_{g}")
                for j in range(4):
                    nc.tensor.matmul(
                        tpo[:, 128 * j:128 * (j + 1)],
                        lhsT=oT_sb[:, 128 * j:128 * (j + 1)], rhs=id_sb,
                        is_transpose=True, start=(j == 0), stop=(j == 3),
                    )
                osb = outp.tile([128, 512], bf16, tag="osb", name=f"osb_{g}", bufs=2)
                osb_r = osb.rearrange("p (j d) -> p j d", j=4)
                if split_dma:
                    # drain path: alternate the scale between ScalarE (free by
                    # now; Copy with a per-partition scale AP) and DVE so the
                    # four strips finish ~2x sooner.
                    Copy = mybir.ActivationFunctionType.Copy
                    for j in range(4):
                        if j % 2 == 0:
                            nc.scalar.activation(
                                osb[:, 128 * j:128 * (j + 1)],
                                tpo[:, 128 * j:128 * (j + 1)],
                                Copy, scale=recip[:, 4 * j:4 * j + 1],
                            )
                        else:
                            nc.vector.tensor_scalar_mul(
                                osb[:, 128 * j:128 * (j + 1)],
                                tpo[:, 128 * j:128 * (j + 1)],
                                recip[:, 4 * j:4 * j + 1],
                            )
                    nc.sync.dma_start(
                        out=out_r[:, 4 * g:4 * (g + 1), :], in_=osb_r,
                    )
                else:
                    for j in range(4):
                        nc.vector.tensor_scalar_mul(
                            osb[:, 128 * j:128 * (j + 1)],
                            tpo[:, 128 * j:128 * (j + 1)],
                            recip[:, 4 * j:4 * j + 1],
                        )
                    nc.sync.dma_start(
                        out=out_r[:, 4 * g:4 * (g + 1), :], in_=osb_r,
                    )

        prologue_kT(0, scalar_eng=True)
        prologue_q(0)
        prologue_at = {
            (0, 1): lambda: prologue_kT(1),
            (0, 2): lambda: prologue_v(0),
            (0, 3): lambda: prologue_kT(2),
            (0, 4): lambda: prologue_v(1),
            (0, 5): lambda: prologue_kT(3),
            (0, 6): lambda: prologue_v(2),
            (0, 7): lambda: prologue_q(1),
            (1, 0): lambda: prologue_v(3),
            (1, 2): lambda: prologue_q(2),
            (2, 2): lambda: prologue_q(3),
        }

        chunks = [(g, c) for g in range(NG) for c in range(NCH)]
        lag = [None, None]  # 2-deep AV lag queue
        for g, c in chunks:
            if (g, c) in prologue_at:
                prologue_at[(g, c)]()
            if c == 0:
                avs[g] = av_p.tile([128, 512], fp32, tag="av", name=f"av_{g}")
            pt = issue_scores(g, c)
            if lag[0] is not None:
                issue_avden(*lag[0])
                if lag[0][1] == NCH - 1:
                    gp = lag[0][0]
                    issue_den_quad(gp, NCH - 2)
                    epilogue_a(gp)
            lag = [lag[1], (g, c, pt)]
            if c == 4 and g > 0:
                epilogue_b(g - 1)
            if c == 6 and g > 0:
                epilogue_c(g - 1)
        # drain: AV for the last two chunks, final den quad, fast epilogue
        for item in lag:
            issue_avden(*item)
        issue_den_quad(NG - 1, NCH - 2)
        epilogue_a(NG - 1, last=True)
        epilogue_b(NG - 1)
        epilogue_c(NG - 1, split_dma=True)

    nc.compile()
    return nc


def _get_program():
    global _PROGRAM
    if _PROGRAM is None:
        _PROGRAM = _build()
    return _PROGRAM


def _ensure_axon_hooks():
    """bass_utils imports antenv.axon_hooks when tracing; provide a stub if
    the image's antenv lacks it (hook defaults to None => tracing skipped)."""
    import sys
    import types
    try:
        import antenv.axon_hooks  # noqa: F401
        return
    except ImportError:
        pass
    import antenv
    m = types.ModuleType("antenv.axon_hooks")
    m._hook = None
    def _set(h):
        m._hook = h
    def _get():
        return m._hook
    m.set_axon_ntff_profile_hook = _set
    m.get_axon_ntff_profile_hook = _get
    sys.modules["antenv.axon_hooks"] = m
    antenv.axon_hooks = m


def kernel(input1, Wq, bq, Wk, bk, Wv, bv):
    global LAST_RESULTS
    _ensure_axon_hooks()
    import ml_dtypes
    from concourse.bass_utils import run_bass_kernel_spmd

    nc = _get_program()
    bft = ml_dtypes.bfloat16

    input1 = np.asarray(input1, dtype=np.float32)
    w3 = np.concatenate([np.asarray(W, np.float32).T for W in (Wq, Wk, Wv)],
                        axis=1).astype(bft)
    sel = np.tile(np.array([1.0 if p % 32 == 0 else 0.0 for p in range(D)],
                  np.float32).reshape(D, 1), (1, 4))
    cb = np.concatenate([
        np.eye(D, dtype=np.float32),
        np.ones((D, 32), np.float32),
        sel,
    ], axis=1).astype(bft)
    cf = np.asarray(bq, np.float32).reshape(D, 1)
    bvb = np.tile(np.asarray(bv, np.float32).reshape(1, D), (D, 4)).astype(bft)
    common = {
        "w3": np.ascontiguousarray(w3),
        "cb": np.ascontiguousarray(cb),
        "cf": np.ascontiguousarray(cf),
        "bvb": np.ascontiguousarray(bvb),
    }
    xb = np.ascontiguousarray(input1.astype(bft))
    in_maps = [dict(common, x=xb[b]) for b in range(8)]
    res = run_bass_kernel_spmd(nc, in_maps, core_ids=list(range(8)))
    LAST_RESULTS = res
    return np.stack([r["out"].astype(np.float32) for r in res.results], axis=0)
